# revision 1
# baseline (speedup 1.0000x reference)
"""Trainium2 Bass kernel for nn_Brain (gnn_message_passing, N=20000, E=20M, B=4, S=8).

Math (faithful to the reference):
    a_0 = zeros(N); a_0[:1000] = x0
    total_u[j] = c0[j] + sum_{d=1..u-1} sum_{e in E_d} w_e * a_{u-d}[from_e]   (to_e = j)
    c0[j]      = sum_{delay-0 edges} w_e * a_0[from_e]     (constant across steps)
    a_u = tanh(total_u), u = 1..8;  output = a_8[-1000:]   (delays >= 8 never fire)

v2 schedule: each delay-d plane is read only ceil((S-d)/d) times instead of once
per step.  A single plane pass uses a batched stationary [128 x 4*ns] holding
snapshots a_slo..a_shi, producing contributions for steps u=slo+d..shi+d in one
PSUM region (partitions 32q..32q+4*ns, banks 0-4).  Total plane traffic drops
from 21 full + 7 partial reads to 14 full + 2 partial (~768MB/core).

Read schedule (after snapshot k becomes available):
  k=1: d1[s1]        (must, ->u2)   d7[s1]   (slip, ->u8, restricted cols)
  k=2: d1[s2] d2[s1..2] (->u3)      d6[s1..2](slip, ->u7,8)
  k=3: d1[s3] d3[s1..3] (->u4)      d5[s1..3](slip, ->u6,7,8)
  k=4: d1[s4] d2[s3..4] d4[s1..4] (->u5)
  k=5: d1[s5]        (->u6)         d3[s4..5](slip, ->u7,8)
  k=6: d1[s6] d2[s5..6] (->u7)
  k=7: d1[s7]        (->u8, restricted cols)
Contributions to u > k+1 are drained by DVE into SBUF accumulators acc4..acc8;
total_u = c0 + direct PSUM slices + acc_u, then tanh (ACT), AllGather, PE
transpose into the f-partitioned snapshot table.
"""
import sys
sys.path.insert(0, '/opt/trn_rl_repo')
import numpy as np
import ml_dtypes

NC_COUNT = 8
WSCALE = 64.0

FULL_CFG = dict(n=20000, e_in=1000, b=4, steps=8, nbank=5, chunk_fb=9, nbuf=4)


def derive(cfg):
    c = dict(cfg)
    n, b, s = c['n'], c['b'], c['steps']
    jp = n // NC_COUNT                      # to-neurons per core (2500)
    jpad = ((jp + 127) // 128) * 128        # 2560
    c.update(
        jp=jp, jpad=jpad,
        lfb=jpad // 128,                    # local from-blocks per core (20)
        nfb=NC_COUNT * (jpad // 128),       # global from-blocks (160)
        fpad=NC_COUNT * jpad,               # padded from-rows (20480)
        nfb0=(c['e_in'] + 127) // 128,      # delay-0 from-blocks (8)
        bank_j=jp // c['nbank'],            # 500
        ntr=jpad // 128,                    # post-gather transpose chunks (20)
    )
    assert jp % c['nbank'] == 0 and c['bank_j'] <= 512
    return c


def _mybir():
    import concourse.mybir as mybir
    return mybir


def make_reads(S):
    """Read schedule: list of dicts in PE issue order."""
    reads = [dict(d=0, slo=0, shi=0, itv=0, must=True, restr=False)]
    TB = {
        1: ([(1, 1, 1)], [(7, 1, 1)]),
        2: ([(1, 2, 2), (2, 1, 2)], [(6, 1, 2)]),
        3: ([(1, 3, 3), (3, 1, 3)], [(5, 1, 3)]),
        4: ([(1, 4, 4), (2, 3, 4), (4, 1, 4)], []),
        5: ([(1, 5, 5)], [(3, 4, 5)]),
        6: ([(1, 6, 6), (2, 5, 6)], []),
        7: ([(1, 7, 7)], []),
    }
    for k in range(1, S):
        musts, slips = TB[k]
        for d, slo, shi in musts:
            assert slo + d == k + 1 and shi == k
            reads.append(dict(d=d, slo=slo, shi=shi, itv=k, must=True,
                              restr=(slo + d == S and shi + d == S)))
        for d, slo, shi in slips:
            assert slo + d > k + 1 and shi == k
            reads.append(dict(d=d, slo=slo, shi=shi, itv=k, must=False,
                              restr=(slo + d == S and shi + d == S)))
    # validate: every (d, u) pair covered exactly once
    seen = set()
    for r in reads[1:]:
        for s in range(r['slo'], r['shi'] + 1):
            key = (r['d'], s + r['d'])
            assert key not in seen and s + r['d'] <= S
            seen.add(key)
    assert seen == {(d, u) for d in range(1, S) for u in range(d + 1, S + 1)}
    return reads


# --------------------------------------------------------------------------
# Bass program
# --------------------------------------------------------------------------
def build_bass(cfg, reps=1):
    from concourse import bass
    mybir = _mybir()
    c = derive(cfg)
    n, b, S = c['n'], c['b'], c['steps']
    jp, jpad, lfb, nfb, nfb0 = c['jp'], c['jpad'], c['lfb'], c['nfb'], c['nfb0']
    nbank, bank_j, chunk_fb = c['nbank'], c['bank_j'], c['chunk_fb']
    NBUF = c['nbuf']
    ntr = c['ntr']
    NSNAP = S - 1
    TGRP = 16
    ngrp = (ntr + TGRP - 1) // TGRP
    e_in = c['e_in']
    jr = jp - e_in                          # restricted col start (1500)
    rbank0 = jr // bank_j                   # first bank of restricted cols (3)
    nbk_r = e_in // bank_j                  # banks in restricted reads (2)
    assert rbank0 * bank_j == jr

    reads = make_reads(S)
    NREADS = len(reads)                     # 17
    NREG = 3                                # rotating PSUM regions @ 32*q (quadrant 3 unusable)

    rhs_elems = chunk_fb * jp               # ring buf bytes per partition (25000)
    w0_ch = max(1, chunk_fb // 4)           # fp32 fb per chunk (2)

    def chunks_of(total, ch):
        out, x = [], 0
        while x < total:
            out.append((x, min(ch, total - x)))
            x += ch
        return out

    # chunk list per rep: (read_idx, f0, ch)
    chunk_list = []
    cum_end = []                            # per-read end chunk index (per rep)
    for ri, r in enumerate(reads):
        ch_list = chunks_of(nfb0, w0_ch) if r['d'] == 0 else chunks_of(nfb, chunk_fb)
        for (f0, ch) in ch_list:
            chunk_list.append((ri, f0, ch))
        cum_end.append(len(chunk_list))
    NCHUNK = len(chunk_list)

    # accumulator targets: contribution (read, s) with u = s+d > itv+1 -> acc_u
    acc_us = sorted({r['slo'] + r['d'] + i
                     for r in reads[1:]
                     for i in range(r['shi'] - r['slo'] + 1)
                     if r['slo'] + r['d'] + i > r['itv'] + 1})
    # per-read future contributions; i>=1 slices sit at partition q+4i which is
    # not 32-aligned (engine APs must start at 0/32/64) -> realign via SBUF
    # staging copy + partition-shifting SBUF->SBUF DMA on the scalar queue.
    for r in reads:
        d, slo, shi, itv = r['d'], r['slo'], r['shi'], r['itv']
        fut = [(i, slo + d + i) for i in range(shi - slo + 1)
               if slo + d + i > itv + 1]
        r['fut_aligned'] = [(i, u) for (i, u) in fut if i == 0]
        r['fut_shift'] = [(i, u) for (i, u) in fut if i >= 1]
    n_shifts = sum(len(r['fut_shift']) for r in reads)
    n_stg = sum(1 for r in reads if r['fut_shift'])
    # ACT drain_sem thresholds per u (python sim of DVE inc order)
    act_thr = {}
    freed = 0
    freed += 1                              # c0 copy
    act_thr[1] = freed
    for k in range(1, S):
        musts = [r for r in reads if r['itv'] == k and r['must']]
        slips = [r for r in reads if r['itv'] == k and not r['must']]
        freed += len(musts)                 # tot op incs by #musts
        act_thr[k + 1] = freed
        freed += len(slips)
    assert freed == NREADS

    nc = bass.Bass(target_bir_lowering=False)

    planes = [nc.declare_dram_parameter(f'w{d}', [128, nfb * jp], mybir.dt.uint8,
                                        isOutput=False) for d in range(1, S)]
    w0_t = nc.declare_dram_parameter('w0', [128, nfb0 * jp], mybir.dt.float32,
                                     isOutput=False)
    a0_t = nc.declare_dram_parameter('a0', [128, nfb0 * b], mybir.dt.float32,
                                     isOutput=False)
    id_t = nc.declare_dram_parameter('ident', [32, 32], mybir.dt.bfloat16,
                                     isOutput=False)
    out_t = nc.declare_dram_parameter('a8', [b, e_in], mybir.dt.float32,
                                      isOutput=True)
    ag_in = nc.dram_tensor('ag_in', [b, jpad], mybir.dt.bfloat16)
    ag_out = nc.dram_tensor('ag_out', [NC_COUNT * b, jpad], mybir.dt.bfloat16)

    from contextlib import ExitStack
    with ExitStack() as _es:
        init_sem = _es.enter_context(nc.semaphore('init_sem'))
        pln_sems = [_es.enter_context(nc.semaphore(f'pln{i}')) for i in range(NBUF)]
        free_sem = _es.enter_context(nc.semaphore('free_sem'))
        drain_sem = _es.enter_context(nc.semaphore('drain_sem'))
        act_sem = _es.enter_context(nc.semaphore('act_sem'))
        agd_sem = _es.enter_context(nc.semaphore('agd_sem'))
        cc_sem = _es.enter_context(nc.semaphore('cc_sem'))
        tr_sem = _es.enter_context(nc.semaphore('tr_sem'))
        cp_sem = _es.enter_context(nc.semaphore('cp_sem'))
        ms_sem = _es.enter_context(nc.semaphore('ms_sem'))
        fin_sem = _es.enter_context(nc.semaphore('fin_sem'))
        stg_sem = _es.enter_context(nc.semaphore('stg_sem'))
        shf_sem = _es.enter_context(nc.semaphore('shf_sem'))
        scr_sem = _es.enter_context(nc.semaphore('scr_sem'))
        sb_rhs = [_es.enter_context(nc.sbuf_tensor(f'sb_rhs{i}', [128, rhs_elems], mybir.dt.uint8))
                  for i in range(NBUF)]
        sb_snap = _es.enter_context(nc.sbuf_tensor('sb_snap', [128, nfb * NSNAP * b], mybir.dt.bfloat16))
        sb_a0 = _es.enter_context(nc.sbuf_tensor('sb_a0', [128, nfb0 * b], mybir.dt.float32))
        sb_id = _es.enter_context(nc.sbuf_tensor('sb_id', [32, 32], mybir.dt.bfloat16))
        sb_c0 = _es.enter_context(nc.sbuf_tensor('sb_c0', [b, jp], mybir.dt.float32))
        sb_tot = _es.enter_context(nc.sbuf_tensor('sb_tot', [b, jp], mybir.dt.float32))
        sb_tmp = _es.enter_context(nc.sbuf_tensor('sb_tmp', [b, jp], mybir.dt.float32))
        sb_tot8 = _es.enter_context(nc.sbuf_tensor('sb_tot8', [b, e_in], mybir.dt.float32))
        sb_a = _es.enter_context(nc.sbuf_tensor('sb_a', [b, jpad], mybir.dt.bfloat16))
        sb_a32 = _es.enter_context(nc.sbuf_tensor('sb_a32', [b, e_in], mybir.dt.float32))
        sb_ag = _es.enter_context(nc.sbuf_tensor('sb_ag', [NC_COUNT * b, jpad], mybir.dt.bfloat16))
        sb_acc = {u: _es.enter_context(nc.sbuf_tensor(
            f'sb_acc{u}', [b, e_in if u == S else jp], mybir.dt.float32))
            for u in acc_us}
        sb_stg = _es.enter_context(nc.sbuf_tensor('sb_stg', [128, jp], mybir.dt.float32))
        sb_scr = _es.enter_context(nc.sbuf_tensor('sb_scr', [b, jp], mybir.dt.float32))
        ps = _es.enter_context(nc.psum_tensor('ps', [128, nbank * 512], mybir.dt.float32))
        psT = _es.enter_context(nc.psum_tensor('psT', [128, 2 * TGRP * NC_COUNT * b], mybir.dt.bfloat16))

        block = _es.enter_context(nc.Block())
        AFT = mybir.ActivationFunctionType
        psr = ps.ap().rearrange('p (k j) -> p k j', k=nbank)
        snapv = sb_snap.ap().rearrange('p (f s b) -> p f s b', f=nfb, s=NSNAP)
        a03 = sb_a0.ap().rearrange('p (f b) -> p f b', f=nfb0)
        psT4 = psT.ap().rearrange('p (h t cb) -> p h t cb', h=2, t=TGRP)
        sb_ag3 = sb_ag.ap().rearrange('cb (k p) -> cb k p', p=128)
        c0_3 = sb_c0.ap().rearrange('p (k j) -> p k j', k=nbank)
        tot_3 = sb_tot.ap().rearrange('p (k j) -> p k j', k=nbank)
        tmp_3 = sb_tmp.ap().rearrange('p (k j) -> p k j', k=nbank)
        tot8_3 = sb_tot8.ap().rearrange('p (k j) -> p k j', k=nbk_r)

        def acc_3(u):
            k = nbk_r if u == S else nbank
            return sb_acc[u].ap().rearrange('p (k j) -> p k j', k=k)

        def reg_q(rg):
            return 32 * (rg % NREG)

        def ps_slice(rg, i, restr_or_u8, full_read):
            """[4, nb, 500] PSUM slice for snapshot-index i of read at region rg.
            restr_or_u8: True when target is the restricted u=8 columns."""
            q = reg_q(rg) + 4 * i
            if not restr_or_u8:
                return psr[q:q + 4, 0:nbank, 0:bank_j]
            if full_read:
                return psr[q:q + 4, rbank0:rbank0 + nbk_r, 0:bank_j]
            return psr[q:q + 4, 0:nbk_r, 0:bank_j]

        # -------------------------------------------- sync: plane DMA stream
        @block.sync
        def _(s):
            s.dma_start(out=sb_a0[:, :], in_=a0_t[:, :]).then_inc(init_sem, 16)
            s.dma_start(out=sb_id[:, :], in_=id_t[:, :]).then_inc(init_sem, 16)
            gi = 0
            for rep in range(reps):
                for (ri, f0, ch) in chunk_list:
                    r = reads[ri]
                    if gi >= NBUF:
                        s.wait_ge(free_sem, gi - NBUF + 1)
                    buf = sb_rhs[gi % NBUF]
                    if r['d'] == 0:
                        src = w0_t.ap().rearrange('p (f j) -> p f j', f=nfb0)[
                            :, f0:f0 + ch, :]
                        dst = buf.ap().bitcast(mybir.dt.float32)[
                            :, 0:ch * jp].rearrange('p (c j) -> p c j', c=ch)
                    elif r['restr']:
                        src = planes[r['d'] - 1].ap().rearrange(
                            'p (f j) -> p f j', f=nfb)[:, f0:f0 + ch, jr:jp]
                        dst = buf[:, 0:ch * e_in].rearrange('p (c j) -> p c j', c=ch)
                    else:
                        src = planes[r['d'] - 1].ap().rearrange(
                            'p (f j) -> p f j', f=nfb)[:, f0:f0 + ch, :]
                        dst = buf[:, 0:ch * jp].rearrange('p (c j) -> p c j', c=ch)
                    s.dma_start(out=dst, in_=src).then_inc(pln_sems[gi % NBUF], 16)
                    gi += 1
                s.wait_ge(act_sem, rep * S + S)
                s.dma_start(out=out_t[:, :], in_=sb_a32[:, :]).then_inc(fin_sem, 16)
            s.wait_ge(fin_sem, 16 * reps)

        # -------------------------------------------- tensor: matmuls + transposes
        @block.tensor
        def _(t):
            ci = 0
            gtr = 0
            rg = 0
            for rep in range(reps):
                for k in range(0, S):
                    if 1 <= k <= S - 1:
                        gs = rep * (S - 1) + k
                        t.wait_ge(agd_sem, 32 * gs)
                        for g in range(ngrp):
                            if gtr >= 2:
                                t.wait_ge(cp_sem, gtr - 1)
                            half = gtr % 2
                            k0 = g * TGRP
                            kcnt = min(TGRP, ntr - k0)
                            for kk in range(kcnt):
                                mm = t.transpose(psT4[:, half, kk, :],
                                                 sb_ag3[:, k0 + kk, :], sb_id[:, :])
                            mm.then_inc(tr_sem, 1)
                            gtr += 1
                    for ri, r in enumerate(reads):
                        if r['itv'] != k:
                            continue
                        d, slo, shi = r['d'], r['slo'], r['shi']
                        ns = shi - slo + 1
                        W = b * ns
                        nbk = nbk_r if r['restr'] else nbank
                        jw = e_in if r['restr'] else jp
                        if rg >= NREG:
                            t.wait_ge(drain_sem, rg - (NREG - 1))
                        if k >= 1:
                            t.wait_ge(cp_sem, (rep * (S - 1) + k) * ngrp)
                        q = reg_q(rg)
                        first_c, last_c = (cum_end[ri - 1] if ri else 0), cum_end[ri]
                        for cj in range(first_c, last_c):
                            _, f0, ch = chunk_list[cj]
                            if ci == 0:
                                t.wait_ge(init_sem, 32)
                            t.wait_ge(pln_sems[ci % NBUF], 16 * (ci // NBUF + 1))
                            buf = sb_rhs[ci % NBUF]
                            if d == 0:
                                rhs3 = buf.ap().bitcast(mybir.dt.float32)[
                                    :, 0:ch * jw].rearrange('p (c j) -> p c j', c=ch)
                            else:
                                rhs3 = buf.ap().bitcast(mybir.dt.float8e4)[
                                    :, 0:ch * jw].rearrange('p (c j) -> p c j', c=ch)
                            for cc in range(ch):
                                fb = f0 + cc
                                if d == 0:
                                    lhsT = a03[:, fb, :]
                                else:
                                    lhsT = snapv[:, fb, slo - 1:shi, :]
                                for bi in range(nbk):
                                    mm = t.matmul(
                                        psr[q:q + W, bi, 0:bank_j],
                                        lhsT,
                                        rhs3[:, cc, bi * bank_j:(bi + 1) * bank_j],
                                        start=(cj == first_c and cc == 0),
                                        stop=(cj == last_c - 1 and cc == ch - 1),
                                        skip_group_check=True)
                            mm.then_inc(free_sem, 1)
                            ci += 1
                        rg += 1

        # static shift plan (same for every rep): interval-ordered
        # stg_plan: read index -> staging seq (per rep); shift_plan: flat list
        stg_seq = {}
        shift_plan = []                      # (ri, i, u, stg_idx)
        for k in range(1, S):
            for ri in range(1, NREADS):
                r = reads[ri]
                if r['itv'] != k or not r['fut_shift']:
                    continue
                stg_seq[ri] = len(stg_seq)
                for (i, u) in r['fut_shift']:
                    shift_plan.append((ri, i, u, stg_seq[ri]))
        assert len(shift_plan) == n_shifts and len(stg_seq) == n_stg

        def stg_cols(u):
            """staging (and scratch) column window for a shifted slice -> acc_u"""
            return (jr, jp) if u == S else (0, jp)

        # -------------------------------------------- scalar: tanh + shift DMAs
        @block.scalar
        def _(a):
            a.wait_ge(ms_sem, 1)
            jshift = 0

            def emit_shifts(a, rep, k, want_must):
                nonlocal jshift
                for (ri, i, uu, si) in shift_plan:
                    if reads[ri]['itv'] != k or reads[ri]['must'] != want_must:
                        continue
                    q = reg_q(rep * NREADS + ri)
                    a.wait_ge(stg_sem, rep * n_stg + si + 1)
                    if jshift >= 1:
                        a.wait_ge(scr_sem, jshift)
                    lo, hi = stg_cols(uu)
                    a.dma_start(
                        out=sb_scr.ap()[0:b, 0:hi - lo],
                        in_=sb_stg.ap()[q + b * i:q + b * i + b, lo:hi],
                    ).then_inc(shf_sem, 16)
                    jshift += 1

            for rep in range(reps):
                for u in range(1, S + 1):
                    if u >= 2:
                        emit_shifts(a, rep, u - 1, True)
                    a.wait_ge(drain_sem, rep * NREADS + act_thr[u])
                    if u == 1:
                        a.activation(sb_a[:, 0:jp], sb_c0[:, :], AFT.Tanh,
                                     scale=1.0 / WSCALE).then_inc(act_sem, 1)
                    elif u < S:
                        a.activation(sb_a[:, 0:jp], sb_tot[:, :], AFT.Tanh,
                                     scale=1.0 / WSCALE).then_inc(act_sem, 1)
                    else:
                        if rep > 0:
                            a.wait_ge(fin_sem, 16 * rep)
                        a.activation(sb_a32[:, :], sb_tot8[:, :], AFT.Tanh,
                                     scale=1.0 / WSCALE).then_inc(act_sem, 1)
                    if u >= 2:
                        emit_shifts(a, rep, u - 1, False)

        # -------------------------------------------- vector: drains + totals + snap copies
        @block.vector
        def _(v):
            gcp = 0
            jadd = 0
            cls_last = {}
            v.memset(sb_a[:, :], 0.0).then_inc(ms_sem, 1)
            for rep in range(reps):
                rbase = rep * NREADS
                acc_started = set()

                def stage_read(ri):
                    """copy whole PSUM region (32-aligned) into sb_stg; the
                    scalar queue then partition-shifts slices i>=1 to sb_scr"""
                    r = reads[ri]
                    ns_r = r['shi'] - r['slo'] + 1
                    q = reg_q(rbase + ri)
                    if q in cls_last:
                        v.wait_ge(shf_sem, 16 * cls_last[q])
                    src = psr[q:q + b * ns_r, 0:nbank, 0:bank_j]
                    dst = sb_stg.ap()[q:q + b * ns_r, 0:jp].rearrange(
                        'p (kk j) -> p kk j', kk=nbank)
                    v.tensor_copy(dst, src).then_inc(stg_sem, 1)
                    last_j = max(jx for jx, (ri2, _, _, _) in
                                 enumerate(shift_plan) if ri2 == ri)
                    cls_last[q] = rep * n_shifts + last_j + 1

                def acc_write(u, src, kk):
                    if u not in acc_started:
                        acc_started.add(u)
                        return v.tensor_copy(acc_3(u)[:, 0:kk, :], src)
                    v.tensor_tensor(tmp_3[:, 0:kk, :],
                                    acc_3(u)[:, 0:kk, :], src,
                                    mybir.AluOpType.add)
                    return v.tensor_copy(acc_3(u)[:, 0:kk, :],
                                         tmp_3[:, 0:kk, :])

                def drain_aligned(ri, inc_drain):
                    """i == 0 future contributions, read PSUM directly"""
                    r = reads[ri]
                    op = None
                    for (i, u) in r['fut_aligned']:
                        src = ps_slice(rbase + ri, i, u == S, not r['restr'])
                        op = acc_write(u, src, nbk_r if u == S else nbank)
                    if inc_drain:
                        assert op is not None
                        op.then_inc(drain_sem, 1)

                def scratch_adds(k):
                    nonlocal jadd
                    for want_must in (True, False):
                        for (ri, i, u, si) in shift_plan:
                            if (reads[ri]['itv'] != k or
                                    reads[ri]['must'] != want_must):
                                continue
                            v.wait_ge(shf_sem, 16 * (jadd + 1))
                            kk = nbk_r if u == S else nbank
                            w = e_in if u == S else jp
                            scr = sb_scr.ap()[0:b, 0:w].rearrange(
                                'p (kk j) -> p kk j', kk=kk)
                            op = acc_write(u, scr, kk)
                            op.then_inc(scr_sem, 1)
                            jadd += 1

                # w0 -> c0
                v.wait_ge(free_sem, rep * NCHUNK + cum_end[0])
                v.tensor_copy(c0_3, ps_slice(rbase + 0, 0, False, True)
                              ).then_inc(drain_sem, 1)
                rg_of = {0: rbase + 0}
                rgl = rbase + 1
                for ri in range(1, NREADS):
                    rg_of[ri] = rgl
                    rgl += 1

                for k in range(1, S):
                    # snapshot copies for snap k
                    for g in range(ngrp):
                        v.wait_ge(tr_sem, gcp + 1)
                        half = gcp % 2
                        k0 = g * TGRP
                        kcnt = min(TGRP, ntr - k0)
                        src = psT4[:, half, 0:kcnt, :].rearrange(
                            'p t (c b) -> p t c b', c=NC_COUNT)
                        dst = sb_snap.ap().rearrange(
                            'p (c kl s b) -> p kl c s b', c=NC_COUNT, s=NSNAP, b=b)[
                            :, k0:k0 + kcnt, :, k - 1, :]
                        v.tensor_copy(dst, src).then_inc(cp_sem, 1)
                        gcp += 1

                    must_ri = [ri for ri in range(1, NREADS)
                               if reads[ri]['itv'] == k and reads[ri]['must']]
                    slip_ri = [ri for ri in range(1, NREADS)
                               if reads[ri]['itv'] == k and not reads[ri]['must']]
                    u = k + 1
                    v.wait_ge(free_sem, rep * NCHUNK + cum_end[must_ri[-1]])
                    # stage must reads with shifted futures (before freeing)
                    for ri in must_ri:
                        assert not reads[ri]['fut_aligned']
                        if reads[ri]['fut_shift']:
                            stage_read(ri)
                    # total_u = c0 + direct slices + acc_u
                    addends = [('c0', None)]
                    for ri in must_ri:
                        addends.append(('ps', ri))
                    if u in sb_acc and u in acc_started:
                        addends.append(('acc', u))
                    A = len(addends)
                    if u < S:
                        c0v, totv, tmpv = c0_3, tot_3, tmp_3
                    else:
                        c0v = c0_3[:, rbank0:rbank0 + nbk_r, :]
                        totv = tot8_3
                        tmpv = tmp_3[:, 0:nbk_r, :]

                    def addend_ap(spec):
                        kind, x = spec
                        if kind == 'c0':
                            return c0v
                        if kind == 'acc':
                            kk = nbk_r if u == S else nbank
                            return acc_3(u)[:, 0:kk, :]
                        return ps_slice(rg_of[x], 0, u == S, not reads[x]['restr'])

                    cur = addend_ap(addends[0])
                    for j in range(A - 1):
                        target = totv if (A - 2 - j) % 2 == 0 else tmpv
                        op = v.tensor_tensor(target, cur, addend_ap(addends[j + 1]),
                                             mybir.AluOpType.add)
                        cur = target
                    op.then_inc(drain_sem, len(must_ri))
                    # slip reads: stage shifted futures, drain aligned ones
                    for ri in slip_ri:
                        v.wait_ge(free_sem, rep * NCHUNK + cum_end[ri])
                        if reads[ri]['fut_shift']:
                            stage_read(ri)
                        drain_aligned(ri, inc_drain=True)
                    scratch_adds(k)

        # -------------------------------------------- gpsimd: allgather chain
        @block.gpsimd
        def _(g):
            for rep in range(reps):
                for u in range(1, S):
                    gs = rep * (S - 1) + u
                    g.wait_ge(act_sem, rep * S + u)
                    g.dma_start(out=ag_in[:, :], in_=sb_a[:, :]).then_inc(agd_sem, 16)
                    g.wait_ge(agd_sem, 32 * gs - 16)
                    g.collective_compute(
                        'AllGather', mybir.AluOpType.bypass,
                        replica_groups=[list(range(NC_COUNT))],
                        ins=[ag_in.ap().opt()], outs=[ag_out.ap().opt()],
                    ).then_inc(cc_sem, 1)
                    g.wait_ge(cc_sem, gs)
                    g.dma_start(out=sb_ag[:, :], in_=ag_out[:, :]).then_inc(agd_sem, 16)
                    g.wait_ge(agd_sem, 32 * gs)

    return nc, c

# --------------------------------------------------------------------------
# Host preprocessing
# --------------------------------------------------------------------------
def preprocess(inputs, cfg):
    c = derive(cfg)
    n, b, S = c['n'], c['b'], c['steps']
    jp, jpad, nfb, nfb0 = c['jp'], c['jpad'], c['nfb'], c['nfb0']
    e_in = c['e_in']

    x0 = np.asarray(inputs['input_data'], np.float32)         # [B, IN]
    fr = np.asarray(inputs['from_idx'], np.int64)
    to = np.asarray(inputs['to_idx'], np.int64)
    dl = np.asarray(inputs['delays'], np.int64)
    w = np.asarray(inputs['connection_weights'], np.float32)

    keep = dl < S
    fr, to, dl, w = fr[keep], to[keep], dl[keep], w[keep]
    # delay-0 edges from f >= e_in contribute 0 forever (a_0 is 0 there)
    keep0 = ~((dl == 0) & (fr >= e_in))
    fr, to, dl, w = fr[keep0], to[keep0], dl[keep0], w[keep0]

    core = to // jp
    jl = to - core * jp
    frow = fr + (jpad - jp) * (fr // jp)      # padded from-row (128-aligned blocks)

    in_maps = [dict() for _ in range(NC_COUNT)]
    for cc in range(NC_COUNT):
        for d in range(S):
            m = (core == cc) & (dl == d)
            if d == 0:
                rows = fr[m]                   # < e_in, no padding shift there
                plane = np.zeros(128 * nfb0 * jp, np.float32)
                np.add.at(plane, (rows % 128) * (nfb0 * jp) +
                          (rows // 128) * jp + jl[m], w[m] * WSCALE)
                in_maps[cc]['w0'] = plane.reshape(128, nfb0 * jp)
            else:
                plane = np.zeros(128 * nfb * jp, np.float32)
                np.add.at(plane, (frow[m] % 128) * (nfb * jp) +
                          (frow[m] // 128) * jp + jl[m], w[m] * WSCALE)
                in_maps[cc][f'w{d}'] = plane.reshape(128, nfb * jp).astype(
                    ml_dtypes.float8_e4m3).view(np.uint8)

    a0 = np.zeros((128, nfb0, b), np.float32)
    for fb in range(nfb0):
        lo, hi = fb * 128, min((fb + 1) * 128, e_in)
        if hi > lo:
            a0[0:hi - lo, fb, :] = x0[:, lo:hi].T
    ident = np.eye(32, dtype=ml_dtypes.bfloat16)
    for cc in range(NC_COUNT):
        in_maps[cc]['a0'] = a0.reshape(128, nfb0 * b)
        in_maps[cc]['ident'] = ident
    return in_maps


# --------------------------------------------------------------------------
# PJRT runner (self-contained)
# --------------------------------------------------------------------------
class Runner:
    def __init__(self, nc, n_cores=NC_COUNT):
        import jax
        from jax.sharding import Mesh, PartitionSpec
        from jax.experimental.shard_map import shard_map
        import concourse.mybir as mybir
        from concourse.bass2jax import (_bass_exec_p, install_neuronx_cc_hook,
                                        partition_id_tensor)
        install_neuronx_cc_hook()
        self.jax = jax
        self.n_cores = n_cores
        partition_name = nc.partition_id_tensor.name if nc.partition_id_tensor else None
        dbg_name = nc.dbg_addr.name if nc.dbg_addr is not None else None
        in_names, out_names, out_avals, zero_outs = [], [], [], []
        for alloc in nc.m.functions[0].allocations:
            if not isinstance(alloc, mybir.MemoryLocationSet):
                continue
            name = alloc.memorylocations[0].name
            if alloc.kind == 'ExternalInput':
                if name not in (partition_name, dbg_name):
                    in_names.append(name)
            elif alloc.kind == 'ExternalOutput':
                out_names.append(name)
                shape = tuple(alloc.tensor_shape)
                dtype = mybir.dt.np(alloc.dtype)
                out_avals.append(jax.core.ShapedArray(shape, dtype))
                zero_outs.append(np.zeros(shape, dtype))
        self.in_names, self.out_names = in_names, out_names
        self.out_avals, self.zero_outs = out_avals, zero_outs
        all_in = list(in_names) + list(out_names)
        if dbg_name is not None:
            all_in.append(dbg_name)
        if partition_name is not None:
            all_in.append(partition_name)
        has_dbg = dbg_name is not None

        def _body(*args):
            operands = list(args)
            if has_dbg:
                operands.append(jax.numpy.zeros((1, 2), jax.numpy.uint32))
            if partition_name is not None:
                operands.append(partition_id_tensor())
            return tuple(_bass_exec_p.bind(
                *operands, out_avals=tuple(out_avals), in_names=tuple(all_in),
                out_names=tuple(out_names), lowering_input_output_aliases=(),
                sim_require_finite=False, sim_require_nnan=False, nc=nc))

        devices = jax.devices()[:n_cores]
        mesh = Mesh(np.asarray(devices), ('core',))
        self._fn = jax.jit(
            shard_map(_body, mesh=mesh,
                      in_specs=(PartitionSpec('core'),) * (len(in_names) + len(out_names)),
                      out_specs=(PartitionSpec('core'),) * len(out_names),
                      check_rep=False),
            keep_unused=True)
        self._sharding = jax.sharding.NamedSharding(mesh, PartitionSpec('core'))

    def put_inputs(self, in_maps):
        jax = self.jax
        dev_in = [jax.device_put(
            np.concatenate([np.asarray(m[name]) for m in in_maps], axis=0),
            self._sharding) for name in self.in_names]
        dev_zero = [jax.device_put(
            np.zeros((self.n_cores * z.shape[0], *z.shape[1:]), z.dtype),
            self._sharding) for z in self.zero_outs]
        return dev_in, dev_zero

    def run(self, dev_in, dev_zero):
        outs = self._fn(*dev_in, *dev_zero)
        self.jax.block_until_ready(outs)
        return outs

    def results(self, outs):
        return [
            {name: np.asarray(outs[i]).reshape(self.n_cores, *self.out_avals[i].shape)[c]
             for i, name in enumerate(self.out_names)}
            for c in range(self.n_cores)
        ]


# --------------------------------------------------------------------------
# public entry point
# --------------------------------------------------------------------------
_CACHE = {}


def _get_runner(cfg_key):
    if cfg_key not in _CACHE:
        cfg = dict(FULL_CFG)
        nc, c = build_bass(cfg)
        _CACHE[cfg_key] = (Runner(nc), c)
    return _CACHE[cfg_key]


def kernel(input_data, from_idx, to_idx, delays, connection_weights, steps):
    assert int(steps) == FULL_CFG['steps']
    runner, c = _get_runner('full')
    in_maps = preprocess(
        dict(input_data=input_data, from_idx=from_idx, to_idx=to_idx,
             delays=delays, connection_weights=connection_weights), FULL_CFG)
    dev_in, dev_zero = runner.put_inputs(in_maps)
    outs = runner.run(dev_in, dev_zero)
    res = runner.results(outs)
    # a_8[-e_in:] lives in core 7's trailing e_in columns == its 'a8' output
    return res[NC_COUNT - 1]['a8'].astype(np.float32)



# revision 9
# speedup vs baseline: 1.6498x; 1.6498x over previous
"""Trainium2 Bass kernel for nn_Brain (gnn_message_passing, N=20000, E=20M, B=4, S=8).

Math (faithful to the reference):
    a_0 = zeros(N); a_0[:1000] = x0
    total_u[j] = c0[j] + sum_{d=1..u-1} sum_{e in E_d} w_e * a_{u-d}[from_e]   (to_e = j)
    c0[j]      = sum_{delay-0 edges} w_e * a_0[from_e]     (constant across steps)
    a_u = tanh(total_u), u = 1..8;  output = a_8[-1000:]   (delays >= 8 never fire)

v4: fp8 DoubleRow matmuls (the v2 PE bottleneck was ~2.7ms at 1 col/cycle;
DoubleRow contracts a 256-row superblock per pass).  Both operands must be
fp8: planes stay fp8e4m3 (x WSCALE), snapshots are stored fp8e4m3 x SSCALE.
w0/a0 move to bf16 (w0 host-scaled by WSCALE*SSCALE); tanh scale =
1/(WSCALE*SSCALE).

DoubleRow ISA restrictions (walrus s3_lw_dual_fp8_restrictions):
  - matmul PSUM dst must start at partition 0 (no column tiling), and
  - stationary free-dim steps beyond the innermost must be 16B-aligned.
So the v2 rotating 32-partition PSUM regions are gone.  Instead each
interval k shares ONE accumulation region at partitions 0..4*(S-k):
row-block (u-k-1)*b holds the merged contribution to step u from all of
this interval's reads.  Row alignment comes from the stationary: the fp8
snapshot table has, per (superblock, ktile), a 20-zero prefix + 28 snapshot
slots (sp=48, so the ktile step 48 is 16B-aligned), and a read with
snapshot range slo..shi and row offset o = slo+d-k-1 loads sp[20+(slo-1-o)*4
: 20+shi*4], contracting zeros into rows below its targets.  At interval
end the DVE stages rows 0..nu_k to a rotating bf16 SBUF buffer and releases
PSUM (PE stall ~2us/interval); totals and the per-(k,u) future shifts
(scalar-queue partition-shift DMAs -> scratch -> acc_u adds) read the
staged copy.  c0 lives at PSUM partitions 32-35 (plain bf16 matmul, legal).

Read schedule (gate = highest snapshot used; gate<k reads run before the
step-k AllGather's transposes and absorb the collective latency):
  k=1:                  d1[s1]
  k=2: d7[s1](g1,restr) d2[s1..2] d1[s2]
  k=3: d6[s1..2](g2)    d3[s1..3] d1[s3]
  k=4: d5[s1..3](g3)    d4[s1..4] d2[s3..4] d1[s4]
  k=5:                  d1[s5]
  k=6: d3[s4..5](g5)    d2[s5..6] d1[s6]
  k=7:                  d1[s7](restr)
Within an interval the first writer of each PSUM bank must cover all rows
consumed from that bank (start=True coverage; validated statically).
"""
import sys
sys.path.insert(0, '/opt/trn_rl_repo')
import numpy as np
import ml_dtypes

NC_COUNT = 8
WSCALE = 64.0
SSCALE = 8.0
ZPFX = 20                                   # zero-prefix slots in sp dim
SPW = 48                                    # sp dim width (20 zeros + 28)

FULL_CFG = dict(n=20000, e_in=1000, b=4, steps=8, nbank=5, chunk_fb=8, nbuf=4,
                nrot=2)


def derive(cfg):
    c = dict(cfg)
    n, b, s = c['n'], c['b'], c['steps']
    jp = n // NC_COUNT                      # to-neurons per core (2500)
    jpad = ((jp + 127) // 128) * 128        # 2560
    c.update(
        jp=jp, jpad=jpad,
        lfb=jpad // 128,                    # local from-blocks per core (20)
        nfb=NC_COUNT * (jpad // 128),       # global from-blocks (160)
        fpad=NC_COUNT * jpad,               # padded from-rows (20480)
        nfb0=(c['e_in'] + 127) // 128,      # delay-0 from-blocks (8)
        bank_j=jp // c['nbank'],            # 500
        ntr=jpad // 128,                    # post-gather transpose chunks (20)
    )
    assert jp % c['nbank'] == 0 and c['bank_j'] <= 512
    assert c['chunk_fb'] % 2 == 0 and c['nfb'] % c['chunk_fb'] == 0
    assert (s - 1) * c['b'] + ZPFX <= SPW and SPW % 16 == 0
    return c


def _mybir():
    import concourse.mybir as mybir
    return mybir


def make_reads(S, b):
    """Read schedule: list of dicts in PE issue order.

    gate = highest snapshot the read uses (PE gates on its copies only).
    o    = row-block offset: snapshot s lands at PSUM rows (s+d-k-1)*b.
    span = o*b + b*ns rows written (zero-prefix rows below the targets).
    """
    reads = [dict(d=0, slo=0, shi=0, itv=0, must=True, restr=False, gate=0,
                  o=0, span=b)]
    TB = {
        1: [(1, 1, 1)],
        2: [(7, 1, 1), (2, 1, 2), (1, 2, 2)],
        3: [(6, 1, 2), (3, 1, 3), (1, 3, 3)],
        4: [(5, 1, 3), (4, 1, 4), (2, 3, 4), (1, 4, 4)],
        5: [(1, 5, 5)],
        6: [(3, 4, 5), (2, 5, 6), (1, 6, 6)],
        7: [(1, 7, 7)],
    }
    for k in range(1, S):
        for d, slo, shi in TB[k]:
            assert shi <= k and slo + d >= k + 1 and shi + d <= S
            o = slo + d - k - 1
            reads.append(dict(d=d, slo=slo, shi=shi, itv=k, gate=shi,
                              must=(slo + d == k + 1), o=o,
                              span=(o + shi - slo + 1) * b,
                              restr=(slo + d == S and shi + d == S)))
    # early (gate<k) reads must precede gated ones (PE emission order is the
    # list order and the threads assume it)
    for k in range(1, S):
        kr = [r for r in reads if r['itv'] == k]
        assert kr == sorted(kr, key=lambda r: r['gate'] >= k)
        # spans must be non-increasing so each bank's first writer covers
        # every row consumed from that bank this interval
        spans = [r['span'] for r in kr]
        assert spans == sorted(spans, reverse=True)
    # coverage: every (d, u) pair exactly once, on time
    seen = set()
    for r in reads[1:]:
        for s in range(r['slo'], r['shi'] + 1):
            key = (r['d'], s + r['d'])
            assert key not in seen and s + r['d'] <= S
            assert s + r['d'] >= r['itv'] + 1
            seen.add(key)
    assert seen == {(d, u) for d in range(1, S) for u in range(d + 1, S + 1)}
    return reads


def interval_info(reads, S, b, nbank, nbk_r):
    """Per-interval: nu (rows staged), consumed (u -> (row0, restr_src)),
    and per-read per-bank start/stop flags."""
    info = {}
    for k in range(1, S):
        kr = [(ri, r) for ri, r in enumerate(reads) if r['itv'] == k]
        nu = max(r['span'] for _, r in kr)
        consumed = {}
        for _, r in kr:
            for i in range(r['shi'] - r['slo'] + 1):
                u = r['slo'] + r['d'] + i
                ro = (r['o'] + i) * b
                prev = consumed.get(u)
                if prev is not None:
                    assert prev == (ro, r['restr']), 'mixed src windows'
                consumed[u] = (ro, r['restr'])
        # per-bank first/last writer -> start/stop flags per read
        first_w, last_w = {}, {}
        for ri, r in kr:
            banks = range(nbk_r) if r['restr'] else range(nbank)
            for bi in banks:
                first_w.setdefault(bi, ri)
                last_w[bi] = ri
        starts, stops = {}, {}
        for ri, r in kr:
            banks = range(nbk_r) if r['restr'] else range(nbank)
            starts[ri] = {bi: (first_w[bi] == ri) for bi in banks}
            stops[ri] = {bi: (last_w[bi] == ri) for bi in banks}
            # coverage check: first writer of each bank spans all consumed
            # rows of that bank this interval
            for bi in banks:
                if first_w[bi] != ri:
                    continue
                for u, (ro, rsrc) in consumed.items():
                    src_banks = range(nbk_r) if rsrc else range(nbank)
                    if bi in src_banks or u == k + 1:
                        assert ro + b <= r['span'], (k, u, bi)
        info[k] = dict(nu=nu, consumed=consumed, starts=starts, stops=stops)
    return info


# --------------------------------------------------------------------------
# Bass program
# --------------------------------------------------------------------------
def build_bass(cfg, reps=1):
    from concourse import bass
    mybir = _mybir()
    c = derive(cfg)
    n, b, S = c['n'], c['b'], c['steps']
    jp, jpad, lfb, nfb, nfb0 = c['jp'], c['jpad'], c['lfb'], c['nfb'], c['nfb0']
    nbank, bank_j, chunk_fb = c['nbank'], c['bank_j'], c['chunk_fb']
    NBUF = c['nbuf']
    NROT = c['nrot']
    ntr = c['ntr']
    NSNAP = S - 1
    TGRP = 16
    ngrp = (ntr + TGRP - 1) // TGRP
    e_in = c['e_in']
    jr = jp - e_in                          # restricted col start (1500)
    rbank0 = jr // bank_j                   # first bank of restricted cols (3)
    nbk_r = e_in // bank_j                  # banks in restricted reads (2)
    assert rbank0 * bank_j == jr

    reads = make_reads(S, b)
    NREADS = len(reads)                     # 17
    iinfo = interval_info(reads, S, b, nbank, nbk_r)

    rhs_elems = chunk_fb * jp               # ring buf bytes per partition (20000)
    w0_ch = chunk_fb // 2                   # bf16 fb per chunk (4)

    def chunks_of(total, ch):
        out, x = [], 0
        while x < total:
            out.append((x, min(ch, total - x)))
            x += ch
        return out

    chunk_list = []
    cum_end = []                            # per-read end chunk index (per rep)
    for ri, r in enumerate(reads):
        ch_list = chunks_of(nfb0, w0_ch) if r['d'] == 0 else chunks_of(nfb, chunk_fb)
        for (f0, ch) in ch_list:
            chunk_list.append((ri, f0, ch))
        cum_end.append(len(chunk_list))
    NCHUNK = len(chunk_list)

    # shift plan: per (k, u>k+1): one partition-shift DMA + one acc add.
    # acc_us: accumulator targets.
    shift_plan = []                         # (k, u, ro, restr_src)
    for k in range(1, S):
        for u in sorted(iinfo[k]['consumed']):
            if u == k + 1:
                continue
            ro, rsrc = iinfo[k]['consumed'][u]
            shift_plan.append((k, u, ro, rsrc))
    n_shifts = len(shift_plan)
    acc_us = sorted({u for (_, u, _, _) in shift_plan})
    # per-u first-shift index (to pick copy vs add into acc) per rep
    first_shift_of_u = {}
    for jx, (k, u, _, _) in enumerate(shift_plan):
        first_shift_of_u.setdefault(u, jx)
    # tanh gating: tot_sem incs are c0-copy (u=1) then one per interval total
    # -> tanh(u) waits tot_sem >= rep*S + u.

    nc = bass.Bass(target_bir_lowering=False)

    planes = [nc.declare_dram_parameter(f'w{d}', [128, nfb * jp], mybir.dt.uint8,
                                        isOutput=False) for d in range(1, S)]
    w0_t = nc.declare_dram_parameter('w0', [128, nfb0 * jp], mybir.dt.bfloat16,
                                     isOutput=False)
    a0_t = nc.declare_dram_parameter('a0', [128, nfb0 * b], mybir.dt.bfloat16,
                                     isOutput=False)
    id_t = nc.declare_dram_parameter('ident', [32, 32], mybir.dt.bfloat16,
                                     isOutput=False)
    out_t = nc.declare_dram_parameter('a8', [b, e_in], mybir.dt.float32,
                                      isOutput=True)
    ag_in = nc.dram_tensor('ag_in', [b, jpad], mybir.dt.bfloat16)
    ag_out = nc.dram_tensor('ag_out', [NC_COUNT * b, jpad], mybir.dt.bfloat16)

    from contextlib import ExitStack
    with ExitStack() as _es:
        init_sem = _es.enter_context(nc.semaphore('init_sem'))
        pln_sems = [_es.enter_context(nc.semaphore(f'pln{i}')) for i in range(NBUF)]
        free_sem = _es.enter_context(nc.semaphore('free_sem'))
        drain_sem = _es.enter_context(nc.semaphore('drain_sem'))
        tot_sem = _es.enter_context(nc.semaphore('tot_sem'))
        act_sem = _es.enter_context(nc.semaphore('act_sem'))
        agd_sem = _es.enter_context(nc.semaphore('agd_sem'))
        cc_sem = _es.enter_context(nc.semaphore('cc_sem'))
        tr_sem = _es.enter_context(nc.semaphore('tr_sem'))
        cp_sem = _es.enter_context(nc.semaphore('cp_sem'))
        ms_sem = _es.enter_context(nc.semaphore('ms_sem'))
        fin_sem = _es.enter_context(nc.semaphore('fin_sem'))
        shf_sem = _es.enter_context(nc.semaphore('shf_sem'))
        scr_sem = _es.enter_context(nc.semaphore('scr_sem'))
        sb_rhs = [_es.enter_context(nc.sbuf_tensor(f'sb_rhs{i}', [128, rhs_elems], mybir.dt.uint8))
                  for i in range(NBUF)]
        sb_snap = _es.enter_context(nc.sbuf_tensor('sb_snap', [128, nfb * SPW], mybir.dt.float8e4))
        sb_a0 = _es.enter_context(nc.sbuf_tensor('sb_a0', [128, nfb0 * b], mybir.dt.bfloat16))
        sb_id = _es.enter_context(nc.sbuf_tensor('sb_id', [32, 32], mybir.dt.bfloat16))
        sb_c0 = _es.enter_context(nc.sbuf_tensor('sb_c0', [b, jp], mybir.dt.float32))
        sb_tot = _es.enter_context(nc.sbuf_tensor('sb_tot', [b, jp], mybir.dt.float32))
        sb_tmp = _es.enter_context(nc.sbuf_tensor('sb_tmp', [b, jp], mybir.dt.float32))
        sb_tot8 = _es.enter_context(nc.sbuf_tensor('sb_tot8', [b, e_in], mybir.dt.float32))
        sb_a = _es.enter_context(nc.sbuf_tensor('sb_a', [b, jpad], mybir.dt.bfloat16))
        sb_a32 = _es.enter_context(nc.sbuf_tensor('sb_a32', [b, e_in], mybir.dt.float32))
        sb_ag = _es.enter_context(nc.sbuf_tensor('sb_ag', [NC_COUNT * b, jpad], mybir.dt.bfloat16))
        sb_acc = {u: _es.enter_context(nc.sbuf_tensor(
            f'sb_acc{u}', [b, e_in if u == S else jp], mybir.dt.float32))
            for u in acc_us}
        # staged PSUM rows (bf16, rotating): totals + shifts read from here
        sb_stg = _es.enter_context(nc.sbuf_tensor('sb_stg', [128, NROT * jp], mybir.dt.bfloat16))
        sb_scr = _es.enter_context(nc.sbuf_tensor('sb_scr', [b, jp], mybir.dt.bfloat16))
        ps = _es.enter_context(nc.psum_tensor('ps', [128, nbank * 512], mybir.dt.float32))
        psT = _es.enter_context(nc.psum_tensor('psT', [128, 2 * TGRP * NC_COUNT * b], mybir.dt.bfloat16))

        block = _es.enter_context(nc.Block())
        AFT = mybir.ActivationFunctionType
        DR = mybir.MatmulPerfMode.DoubleRow
        psr = ps.ap().rearrange('p (k j) -> p k j', k=nbank)
        # snapshot table: [p, superblock f2, ktile t, sp] with 20-zero prefix
        snapv = sb_snap.ap().rearrange('p (f2 t sp) -> p f2 t sp',
                                       f2=nfb // 2, t=2)
        a03 = sb_a0.ap().rearrange('p (f b) -> p f b', f=nfb0)
        psT4 = psT.ap().rearrange('p (h t cb) -> p h t cb', h=2, t=TGRP)
        sb_ag3 = sb_ag.ap().rearrange('cb (k p) -> cb k p', p=128)
        c0_3 = sb_c0.ap().rearrange('p (k j) -> p k j', k=nbank)
        tot_3 = sb_tot.ap().rearrange('p (k j) -> p k j', k=nbank)
        tmp_3 = sb_tmp.ap().rearrange('p (k j) -> p k j', k=nbank)
        tot8_3 = sb_tot8.ap().rearrange('p (k j) -> p k j', k=nbk_r)
        stg4 = sb_stg.ap().rearrange('p (r k j) -> p r k j', r=NROT, k=nbank)

        def acc_3(u):
            k = nbk_r if u == S else nbank
            return sb_acc[u].ap().rearrange('p (k j) -> p k j', k=k)

        def rot_of(k_abs):
            return k_abs % NROT

        # -------------------------------------------- sync: plane DMA stream
        @block.sync
        def _(s):
            s.dma_start(out=sb_a0[:, :], in_=a0_t[:, :]).then_inc(init_sem, 16)
            s.dma_start(out=sb_id[:, :], in_=id_t[:, :]).then_inc(init_sem, 16)
            gi = 0
            for rep in range(reps):
                for (ri, f0, ch) in chunk_list:
                    r = reads[ri]
                    if gi >= NBUF:
                        s.wait_ge(free_sem, gi - NBUF + 1)
                    buf = sb_rhs[gi % NBUF]
                    if r['d'] == 0:
                        src = w0_t.ap().rearrange('p (f j) -> p f j', f=nfb0)[
                            :, f0:f0 + ch, :]
                        dst = buf.ap().bitcast(mybir.dt.bfloat16)[
                            :, 0:ch * jp].rearrange('p (c j) -> p c j', c=ch)
                    elif r['restr']:
                        src = planes[r['d'] - 1].ap().rearrange(
                            'p (f j) -> p f j', f=nfb)[:, f0:f0 + ch, jr:jp]
                        dst = buf[:, 0:ch * e_in].rearrange('p (c j) -> p c j', c=ch)
                    else:
                        src = planes[r['d'] - 1].ap().rearrange(
                            'p (f j) -> p f j', f=nfb)[:, f0:f0 + ch, :]
                        dst = buf[:, 0:ch * jp].rearrange('p (c j) -> p c j', c=ch)
                    s.dma_start(out=dst, in_=src).then_inc(pln_sems[gi % NBUF], 16)
                    gi += 1
                s.wait_ge(act_sem, rep * S + S)
                s.dma_start(out=out_t[:, :], in_=sb_a32[:, :]).then_inc(fin_sem, 16)
            s.wait_ge(fin_sem, 16 * reps)

        # -------------------------------------------- tensor: matmuls + transposes
        @block.tensor
        def _(t):
            ci = 0
            gtr = 0

            def emit_read(ri, rep, k):
                nonlocal ci
                r = reads[ri]
                d, slo, shi, o = r['d'], r['slo'], r['shi'], r['o']
                nbk = nbk_r if r['restr'] else nbank
                jw = e_in if r['restr'] else jp
                span = r['span']
                if r['gate'] >= 1:
                    t.wait_ge(cp_sem, (rep * (S - 1) + r['gate']) * ngrp)
                starts = iinfo[k]['starts'][ri] if k >= 1 else None
                stops = iinfo[k]['stops'][ri] if k >= 1 else None
                first_c, last_c = (cum_end[ri - 1] if ri else 0), cum_end[ri]
                for cj in range(first_c, last_c):
                    _, f0, ch = chunk_list[cj]
                    if ci == 0:
                        t.wait_ge(init_sem, 32)
                    t.wait_ge(pln_sems[ci % NBUF], 16 * (ci // NBUF + 1))
                    buf = sb_rhs[ci % NBUF]
                    if d == 0:
                        rhs3 = buf.ap().bitcast(mybir.dt.bfloat16)[
                            :, 0:ch * jw].rearrange('p (c j) -> p c j', c=ch)
                        for cc in range(ch):
                            lhsT = a03[:, f0 + cc, :]
                            for bi in range(nbk):
                                mm = t.matmul(
                                    psr[32:32 + b, bi, 0:bank_j],
                                    lhsT,
                                    rhs3[:, cc, bi * bank_j:(bi + 1) * bank_j],
                                    start=(cj == first_c and cc == 0),
                                    stop=(cj == last_c - 1 and cc == ch - 1),
                                    skip_group_check=True)
                    else:
                        # DoubleRow: [p, 2, jw] moving vs [p, 2, sp-slice] fp8
                        rhs4 = buf.ap().bitcast(mybir.dt.float8e4)[
                            :, 0:ch * jw].rearrange('p (c2 t j) -> p c2 t j',
                                                    c2=ch // 2, t=2)
                        sp0 = ZPFX + (slo - 1 - o) * b
                        for cc2 in range(ch // 2):
                            f2 = (f0 + 2 * cc2) // 2
                            lhsT = snapv[:, f2, :, sp0:sp0 + span]
                            for bi in range(nbk):
                                mm = t.matmul(
                                    psr[0:span, bi, 0:bank_j],
                                    lhsT,
                                    rhs4[:, cc2, :, bi * bank_j:(bi + 1) * bank_j],
                                    start=(starts[bi] and cj == first_c and cc2 == 0),
                                    stop=(stops[bi] and cj == last_c - 1
                                          and cc2 == ch // 2 - 1),
                                    perf_mode=DR,
                                    skip_group_check=True)
                    mm.then_inc(free_sem, 1)
                    ci += 1

            for rep in range(reps):
                for k in range(0, S):
                    if k >= 1:
                        # region reuse: stages of intervals 1..k-1 (this rep)
                        # and all of the previous rep must be done
                        t.wait_ge(drain_sem, rep * (S - 1) + k - 1)
                    else:
                        if rep > 0:
                            # c0 PSUM region (parts 32+) reuse: previous rep's
                            # c0 copy must be done
                            t.wait_ge(tot_sem, (rep - 1) * S + 1)
                    for ri, r in enumerate(reads):
                        if r['itv'] == k and r['gate'] < k:
                            emit_read(ri, rep, k)
                    if 1 <= k <= S - 1:
                        gs = rep * (S - 1) + k
                        t.wait_ge(agd_sem, 32 * gs)
                        for g in range(ngrp):
                            if gtr >= 2:
                                t.wait_ge(cp_sem, gtr - 1)
                            half = gtr % 2
                            k0 = g * TGRP
                            kcnt = min(TGRP, ntr - k0)
                            for kk in range(kcnt):
                                mm = t.transpose(psT4[:, half, kk, :],
                                                 sb_ag3[:, k0 + kk, :], sb_id[:, :])
                            mm.then_inc(tr_sem, 1)
                            gtr += 1
                    for ri, r in enumerate(reads):
                        if r['itv'] == k and r['gate'] >= k:
                            emit_read(ri, rep, k)

        # -------------------------------------------- scalar: tanh + shift DMAs
        @block.scalar
        def _(a):
            a.wait_ge(ms_sem, 2)
            jshift = 0
            for rep in range(reps):
                for u in range(1, S + 1):
                    a.wait_ge(tot_sem, rep * S + u)
                    if u == 1:
                        a.activation(sb_a[:, 0:jp], sb_c0[:, :], AFT.Tanh,
                                     scale=1.0 / (WSCALE * SSCALE)).then_inc(act_sem, 1)
                    elif u < S:
                        a.activation(sb_a[:, 0:jp], sb_tot[:, :], AFT.Tanh,
                                     scale=1.0 / (WSCALE * SSCALE)).then_inc(act_sem, 1)
                    else:
                        if rep > 0:
                            a.wait_ge(fin_sem, 16 * rep)
                        a.activation(sb_a32[:, :], sb_tot8[:, :], AFT.Tanh,
                                     scale=1.0 / (WSCALE * SSCALE)).then_inc(act_sem, 1)
                    # interval k=u-1's future shifts: stg -> scratch
                    if u <= S - 1:
                        k = u - 1
                        for (kk2, uu, ro, rsrc) in shift_plan:
                            if kk2 != k:
                                continue
                            # stage(k) done is implied by tot_sem(u) above
                            if jshift >= 1:
                                a.wait_ge(scr_sem, jshift)
                            nbs = nbk_r if uu == S else nbank
                            blo = 0 if (uu < S or rsrc) else rbank0
                            w = nbs * bank_j
                            rr = rot_of(rep * (S - 1) + k - 1)
                            a.dma_start(
                                out=sb_scr.ap()[0:b, 0:w],
                                in_=stg4[ro:ro + b, rr, blo:blo + nbs, :],
                            ).then_inc(shf_sem, 16)
                            jshift += 1

        # -------------------------------------------- vector: stage + totals + snap copies
        @block.vector
        def _(v):
            gcp = 0
            jadd = 0
            cls_last = {}
            v.memset(sb_a[:, :], 0.0).then_inc(ms_sem, 1)
            v.memset(sb_snap[:, :], 0.0).then_inc(ms_sem, 1)
            for rep in range(reps):

                def acc_write(u, src, kk, jx):
                    if jx == first_shift_of_u[u]:
                        return v.tensor_copy(acc_3(u)[:, 0:kk, :], src)
                    v.tensor_tensor(tmp_3[:, 0:kk, :],
                                    acc_3(u)[:, 0:kk, :], src,
                                    mybir.AluOpType.add)
                    return v.tensor_copy(acc_3(u)[:, 0:kk, :],
                                         tmp_3[:, 0:kk, :])

                # w0 -> c0 (PSUM partitions 32..32+b)
                v.wait_ge(free_sem, rep * NCHUNK + cum_end[0])
                v.tensor_copy(c0_3, psr[32:32 + b, 0:nbank, 0:bank_j]
                              ).then_inc(tot_sem, 1)

                for k in range(1, S):
                    # snapshot-k copies (scaled bf16 -> fp8)
                    for g in range(ngrp):
                        v.wait_ge(tr_sem, gcp + 1)
                        half = gcp % 2
                        k0 = g * TGRP
                        kcnt = min(TGRP, ntr - k0)
                        src = psT4[:, half, 0:kcnt, :].rearrange(
                            'p t (c b) -> p t c b', c=NC_COUNT)
                        dst = sb_snap.ap().rearrange(
                            'p (c kl sp) -> p kl c sp', c=NC_COUNT, sp=SPW)[
                            :, k0:k0 + kcnt, :,
                            ZPFX + (k - 1) * b:ZPFX + k * b]
                        v.tensor_scalar_mul(dst, src, SSCALE).then_inc(cp_sem, 1)
                        gcp += 1

                    u = k + 1
                    nu = iinfo[k]['nu']
                    last_ri = max(ri for ri, r in enumerate(reads)
                                  if r['itv'] == k)
                    v.wait_ge(free_sem, rep * NCHUNK + cum_end[last_ri])
                    # stage rows 0..nu to the rotating bf16 buffer, release PSUM
                    rr = rot_of(rep * (S - 1) + k - 1)
                    if rr in cls_last:
                        v.wait_ge(shf_sem, 16 * cls_last[rr])
                    v.tensor_copy(stg4[0:nu, rr, :, :],
                                  psr[0:nu, 0:nbank, 0:bank_j]
                                  ).then_inc(drain_sem, 1)
                    lastj = [jx for jx, (kk2, _, _, _) in enumerate(shift_plan)
                             if kk2 == k]
                    if lastj:
                        cls_last[rr] = rep * n_shifts + lastj[-1] + 1
                    # total_u = c0 + stg rows 0..b + acc_u
                    direct_restr = reads[last_ri]['restr'] and u == S
                    addends = [('c0', None), ('stg', rr)]
                    # every shift (k', u) has k' <= u-2 = k-1, so acc_u is
                    # complete by now whenever it exists at all
                    if u in sb_acc:
                        addends.append(('acc', u))
                    A = len(addends)
                    if u < S:
                        c0v, totv, tmpv = c0_3, tot_3, tmp_3
                        stgv = stg4[0:b, rr, 0:nbank, :]
                    else:
                        c0v = c0_3[:, rbank0:rbank0 + nbk_r, :]
                        totv = tot8_3
                        tmpv = tmp_3[:, 0:nbk_r, :]
                        blo = 0 if direct_restr else rbank0
                        stgv = stg4[0:b, rr, blo:blo + nbk_r, :]

                    def addend_ap(spec):
                        kind, x = spec
                        if kind == 'c0':
                            return c0v
                        if kind == 'stg':
                            return stgv
                        kk = nbk_r if u == S else nbank
                        return acc_3(u)[:, 0:kk, :]

                    cur = addend_ap(addends[0])
                    op = None
                    for j in range(A - 1):
                        target = totv if (A - 2 - j) % 2 == 0 else tmpv
                        op = v.tensor_tensor(target, cur, addend_ap(addends[j + 1]),
                                             mybir.AluOpType.add)
                        cur = target
                    op.then_inc(tot_sem, 1)
                    # scratch adds for interval k's shifts
                    for jx, (kk2, uu, ro, rsrc) in enumerate(shift_plan):
                        if kk2 != k:
                            continue
                        v.wait_ge(shf_sem, 16 * (jadd + 1))
                        kk = nbk_r if uu == S else nbank
                        w = e_in if uu == S else jp
                        scr = sb_scr.ap()[0:b, 0:w].rearrange(
                            'p (kk j) -> p kk j', kk=kk)
                        op2 = acc_write(uu, scr, kk, jx)
                        op2.then_inc(scr_sem, 1)
                        jadd += 1

        # -------------------------------------------- gpsimd: allgather chain
        @block.gpsimd
        def _(g):
            for rep in range(reps):
                for u in range(1, S):
                    gs = rep * (S - 1) + u
                    g.wait_ge(act_sem, rep * S + u)
                    g.dma_start(out=ag_in[:, :], in_=sb_a[:, :]).then_inc(agd_sem, 16)
                    g.wait_ge(agd_sem, 32 * gs - 16)
                    g.collective_compute(
                        'AllGather', mybir.AluOpType.bypass,
                        replica_groups=[list(range(NC_COUNT))],
                        ins=[ag_in.ap().opt()], outs=[ag_out.ap().opt()],
                    ).then_inc(cc_sem, 1)
                    g.wait_ge(cc_sem, gs)
                    g.dma_start(out=sb_ag[:, :], in_=ag_out[:, :]).then_inc(agd_sem, 16)
                    g.wait_ge(agd_sem, 32 * gs)

    return nc, c

# --------------------------------------------------------------------------
# Host preprocessing
# --------------------------------------------------------------------------
def preprocess(inputs, cfg):
    c = derive(cfg)
    n, b, S = c['n'], c['b'], c['steps']
    jp, jpad, nfb, nfb0 = c['jp'], c['jpad'], c['nfb'], c['nfb0']
    e_in = c['e_in']

    x0 = np.asarray(inputs['input_data'], np.float32)         # [B, IN]
    fr = np.asarray(inputs['from_idx'], np.int64)
    to = np.asarray(inputs['to_idx'], np.int64)
    dl = np.asarray(inputs['delays'], np.int64)
    w = np.asarray(inputs['connection_weights'], np.float32)

    keep = dl < S
    fr, to, dl, w = fr[keep], to[keep], dl[keep], w[keep]
    # delay-0 edges from f >= e_in contribute 0 forever (a_0 is 0 there)
    keep0 = ~((dl == 0) & (fr >= e_in))
    fr, to, dl, w = fr[keep0], to[keep0], dl[keep0], w[keep0]

    core = to // jp
    jl = to - core * jp
    frow = fr + (jpad - jp) * (fr // jp)      # padded from-row (128-aligned blocks)

    in_maps = [dict() for _ in range(NC_COUNT)]
    for cc in range(NC_COUNT):
        for d in range(S):
            m = (core == cc) & (dl == d)
            if d == 0:
                rows = fr[m]                   # < e_in, no padding shift there
                plane = np.zeros(128 * nfb0 * jp, np.float32)
                np.add.at(plane, (rows % 128) * (nfb0 * jp) +
                          (rows // 128) * jp + jl[m], w[m] * WSCALE * SSCALE)
                in_maps[cc]['w0'] = plane.reshape(128, nfb0 * jp).astype(
                    ml_dtypes.bfloat16)
            else:
                plane = np.zeros(128 * nfb * jp, np.float32)
                np.add.at(plane, (frow[m] % 128) * (nfb * jp) +
                          (frow[m] // 128) * jp + jl[m], w[m] * WSCALE)
                in_maps[cc][f'w{d}'] = plane.reshape(128, nfb * jp).astype(
                    ml_dtypes.float8_e4m3).view(np.uint8)

    a0 = np.zeros((128, nfb0, b), np.float32)
    for fb in range(nfb0):
        lo, hi = fb * 128, min((fb + 1) * 128, e_in)
        if hi > lo:
            a0[0:hi - lo, fb, :] = x0[:, lo:hi].T
    ident = np.eye(32, dtype=ml_dtypes.bfloat16)
    for cc in range(NC_COUNT):
        in_maps[cc]['a0'] = a0.reshape(128, nfb0 * b).astype(ml_dtypes.bfloat16)
        in_maps[cc]['ident'] = ident
    return in_maps


# --------------------------------------------------------------------------
# PJRT runner (self-contained)
# --------------------------------------------------------------------------
class Runner:
    def __init__(self, nc, n_cores=NC_COUNT):
        import jax
        from jax.sharding import Mesh, PartitionSpec
        from jax.experimental.shard_map import shard_map
        import concourse.mybir as mybir
        from concourse.bass2jax import (_bass_exec_p, install_neuronx_cc_hook,
                                        partition_id_tensor)
        install_neuronx_cc_hook()
        self.jax = jax
        self.n_cores = n_cores
        partition_name = nc.partition_id_tensor.name if nc.partition_id_tensor else None
        dbg_name = nc.dbg_addr.name if nc.dbg_addr is not None else None
        in_names, out_names, out_avals, zero_outs = [], [], [], []
        for alloc in nc.m.functions[0].allocations:
            if not isinstance(alloc, mybir.MemoryLocationSet):
                continue
            name = alloc.memorylocations[0].name
            if alloc.kind == 'ExternalInput':
                if name not in (partition_name, dbg_name):
                    in_names.append(name)
            elif alloc.kind == 'ExternalOutput':
                out_names.append(name)
                shape = tuple(alloc.tensor_shape)
                dtype = mybir.dt.np(alloc.dtype)
                out_avals.append(jax.core.ShapedArray(shape, dtype))
                zero_outs.append(np.zeros(shape, dtype))
        self.in_names, self.out_names = in_names, out_names
        self.out_avals, self.zero_outs = out_avals, zero_outs
        all_in = list(in_names) + list(out_names)
        if dbg_name is not None:
            all_in.append(dbg_name)
        if partition_name is not None:
            all_in.append(partition_name)
        has_dbg = dbg_name is not None

        def _body(*args):
            operands = list(args)
            if has_dbg:
                operands.append(jax.numpy.zeros((1, 2), jax.numpy.uint32))
            if partition_name is not None:
                operands.append(partition_id_tensor())
            return tuple(_bass_exec_p.bind(
                *operands, out_avals=tuple(out_avals), in_names=tuple(all_in),
                out_names=tuple(out_names), lowering_input_output_aliases=(),
                sim_require_finite=False, sim_require_nnan=False, nc=nc))

        devices = jax.devices()[:n_cores]
        mesh = Mesh(np.asarray(devices), ('core',))
        self._fn = jax.jit(
            shard_map(_body, mesh=mesh,
                      in_specs=(PartitionSpec('core'),) * (len(in_names) + len(out_names)),
                      out_specs=(PartitionSpec('core'),) * len(out_names),
                      check_rep=False),
            keep_unused=True)
        self._sharding = jax.sharding.NamedSharding(mesh, PartitionSpec('core'))

    def put_inputs(self, in_maps):
        jax = self.jax
        dev_in = [jax.device_put(
            np.concatenate([np.asarray(m[name]) for m in in_maps], axis=0),
            self._sharding) for name in self.in_names]
        dev_zero = [jax.device_put(
            np.zeros((self.n_cores * z.shape[0], *z.shape[1:]), z.dtype),
            self._sharding) for z in self.zero_outs]
        return dev_in, dev_zero

    def run(self, dev_in, dev_zero):
        outs = self._fn(*dev_in, *dev_zero)
        self.jax.block_until_ready(outs)
        return outs

    def results(self, outs):
        return [
            {name: np.asarray(outs[i]).reshape(self.n_cores, *self.out_avals[i].shape)[c]
             for i, name in enumerate(self.out_names)}
            for c in range(self.n_cores)
        ]


# --------------------------------------------------------------------------
# public entry point
# --------------------------------------------------------------------------
_CACHE = {}


def _get_runner(cfg_key):
    if cfg_key not in _CACHE:
        cfg = dict(FULL_CFG)
        nc, c = build_bass(cfg)
        _CACHE[cfg_key] = (Runner(nc), c)
    return _CACHE[cfg_key]


def kernel(input_data, from_idx, to_idx, delays, connection_weights, steps):
    assert int(steps) == FULL_CFG['steps']
    runner, c = _get_runner('full')
    in_maps = preprocess(
        dict(input_data=input_data, from_idx=from_idx, to_idx=to_idx,
             delays=delays, connection_weights=connection_weights), FULL_CFG)
    dev_in, dev_zero = runner.put_inputs(in_maps)
    outs = runner.run(dev_in, dev_zero)
    res = runner.results(outs)
    # a_8[-e_in:] lives in core 7's trailing e_in columns == its 'a8' output
    return res[NC_COUNT - 1]['a8'].astype(np.float32)
